# revision 5
# baseline (speedup 1.0000x reference)
"""Trainium2 Bass kernel for BiLSTM-CRF (B=64, T=512, D=768, H=384, K=9).

Sharding: 8-way data parallel over batch (b=8 per core). All compute on
device; host only reshapes/shards inputs and concatenates outputs.

Per-core layouts (everything "transposed": feature dim on partitions,
(t, batch) in the free dim) so elementwise LSTM-cell work uses all 128
partitions and the recurrent state needs no per-step transposes:
  xT      [768, T*b]   layer-0 input
  WxT     [12*128, cols] gate pre-activations (gate-chunk-major)
  hT      [384, T*b]   per-direction hidden history (DRAM staged)
  featsT  [9, T*b] -> transposed to [(t,b), 9] for LayerNorm + CRF
Gate order is host-remapped to (i, f, o, g) so sigmoid covers one
contiguous [128, 72] slab and tanh one [128, 24] slab per step.
CRF runs sequentially (T steps) on 8 partitions; Viterbi backpointers are
encoded as (argmax - 1e5) floats so ties resolve to the first index,
matching jnp.argmax.
"""

import numpy as np

B, T, D = 64, 512, 768
H = 384
K = 9
START = 7
NEG = -10000.0
BIGF = 100000.0  # index-encoding offset (exact in fp32 for small ints)
NCORES = 8
BLOC = B // NCORES  # 8
TC = 64             # time-chunk for the LSTM scans
NCH = T // TC
GD = 4 * H          # 1536
NM = GD // 128      # 12 gate chunks
NKD = D // 128      # 6 input contraction chunks
NKH = H // 128      # 3 recurrent contraction chunks
CT = T * BLOC       # 4096 columns

_cache = {}


def _build_nc():
    import concourse.bass as bass
    import concourse.mybir as mybir
    from concourse.tile import TileContext
    from concourse.bass import broadcast_tensor_aps

    def bc(full, small):
        _, s2 = broadcast_tensor_aps(full, small)
        return s2

    fp32 = mybir.dt.float32
    AF = mybir.ActivationFunctionType
    ALU = mybir.AluOpType
    AX = mybir.AxisListType

    nc = bass.Bass()

    def par(name, shape, out=False):
        return nc.declare_dram_parameter(name, list(shape), fp32, isOutput=out)

    xT = par("xT", [D, CT])
    wih = {u: par(f"wih_{u}", [128, NKD * GD]) for u in ("l0f", "l0b", "l1f", "l1b")}
    whh = {u: par(f"whh_{u}", [128, NKH * GD]) for u in ("l0f", "l0b", "l1f", "l1b")}
    bia = {u: par(f"bias_{u}", [128, NM]) for u in ("l0f", "l0b", "l1f", "l1b")}
    fcw = par("fcw", [128, NKD * K])
    fcb = par("fcb", [K, 1])
    lng = par("lng_rep", [128, 32 * K])
    lnb = par("lnb_rep", [128, 32 * K])
    trep = par("trans_rep", [BLOC, K * K])
    io81 = par("iota81", [BLOC, K * K])
    io9 = par("iota9", [BLOC, K])
    la0p = par("la0", [BLOC, K])
    id9 = par("ident9", [K, K])

    hT = {u: nc.dram_tensor(f"hT_{u}", [H, CT], fp32) for u in ("l0f", "l0b", "l1f", "l1b")}
    feats_d = nc.dram_tensor("feats_d", [T * BLOC, K], fp32)

    out_score = par("out_score", [BLOC, 1], out=True)
    out_path = par("out_path", [BLOC, T], out=True)

    with TileContext(nc) as tc:
        # ---------------- LSTM scans ----------------
        with (
            tc.tile_pool(name="wpool", bufs=1) as wpool,
            tc.tile_pool(name="xpool", bufs=2) as xpool,
            tc.tile_pool(name="wxpool", bufs=2) as wxpool,
            tc.tile_pool(name="hpool", bufs=2) as hpool,
            tc.tile_pool(name="cellpool", bufs=3) as cellpool,
            tc.tile_pool(name="gpsum", bufs=2, space="PSUM") as gpsum,
            tc.tile_pool(name="rpsum", bufs=4, space="PSUM") as rpsum,
        ):
            def scan_unit(u, srcs, fwd):
                wih_sb = wpool.tile([128, NKD * GD], fp32, tag="wih")
                nc.sync.dma_start(out=wih_sb[:], in_=wih[u][:])
                whh_sb = wpool.tile([128, NKH * GD], fp32, tag="whh")
                nc.sync.dma_start(out=whh_sb[:], in_=whh[u][:])
                bias_sb = wpool.tile([128, NM], fp32, tag="bias")
                nc.sync.dma_start(out=bias_sb[:], in_=bia[u][:])

                h_prev_tile = None
                c_prev = None
                for c in range(NCH):
                    cb = (c if fwd else (NCH - 1 - c)) * TC * BLOC  # column base
                    # ---- input GEMM for this chunk ----
                    xt = xpool.tile([128, NKD * TC * BLOC], fp32, tag="xc")
                    col = 0
                    for (src, nk) in srcs:
                        sv = src.rearrange("(k p) t -> p k t", p=128)
                        nc.sync.dma_start(
                            out=xt[:, col * TC * BLOC:(col + nk) * TC * BLOC]
                            .rearrange("p (k c) -> p k c", k=nk),
                            in_=sv[:, :, cb:cb + TC * BLOC],
                        )
                        col += nk
                    wx = wxpool.tile([128, NM * TC * BLOC], fp32, tag="wx")
                    for m in range(NM):
                        ps = gpsum.tile([128, TC * BLOC], fp32, tag="gps")
                        for k in range(NKD):
                            nc.tensor.matmul(
                                ps[:],
                                wih_sb[:, k * GD + m * 128:k * GD + (m + 1) * 128],
                                xt[:, k * TC * BLOC:(k + 1) * TC * BLOC],
                                start=(k == 0), stop=(k == NKD - 1),
                            )
                        dst = wx[:, m * TC * BLOC:(m + 1) * TC * BLOC]
                        if m % 2 == 0:
                            nc.vector.tensor_scalar_add(dst, ps[:], bias_sb[:, m:m + 1])
                        else:
                            nc.scalar.activation(dst, ps[:], AF.Identity,
                                                 bias=bias_sb[:, m:m + 1])

                    # ---- recurrence over TC steps ----
                    SL = TC + 2  # per-hc slots: [carry_fwd, steps..., carry_bwd]
                    h_sb = hpool.tile([128, NKH * SL * BLOC], fp32, tag="hsb")
                    if c == 0:
                        s0 = 0 if fwd else SL - 1
                        z = h_sb.rearrange("p (h s c) -> p h s c", h=NKH, c=BLOC)
                        nc.vector.memset(z[:, :, s0, :], 0.0)
                        c_prev = cellpool.tile([128, NKH * BLOC], fp32, tag="cst")
                        nc.vector.memset(c_prev[:], 0.0)
                    else:
                        # carry h from previous chunk
                        pv = h_prev_tile.rearrange("p (h s c) -> p h s c", h=NKH, c=BLOC)
                        zv = h_sb.rearrange("p (h s c) -> p h s c", h=NKH, c=BLOC)
                        if fwd:
                            nc.vector.tensor_copy(zv[:, :, 0, :], pv[:, :, TC, :])
                        else:
                            nc.vector.tensor_copy(zv[:, :, SL - 1, :], pv[:, :, 1, :])
                    h_prev_tile = h_sb

                    for j in range(TC):
                        rs = j if fwd else (SL - 1 - j)          # read slot
                        ws = j + 1 if fwd else (SL - 2 - j)      # write slot
                        lcol = (j if fwd else (TC - 1 - j)) * BLOC
                        ps = rpsum.tile([128, NM * BLOC], fp32, tag="rps")
                        for m in range(NM):
                            for k in range(NKH):
                                nc.tensor.matmul(
                                    ps[:, m * BLOC:(m + 1) * BLOC],
                                    whh_sb[:, k * GD + m * 128:k * GD + (m + 1) * 128],
                                    h_sb[:, (k * SL + rs) * BLOC:(k * SL + rs + 1) * BLOC],
                                    start=(k == 0), stop=(k == NKH - 1),
                                )
                        g = cellpool.tile([128, NM * BLOC], fp32, tag="g")
                        nc.vector.tensor_add(
                            g.rearrange("p (m c) -> p m c", m=NM),
                            ps.rearrange("p (m c) -> p m c", m=NM),
                            wx.rearrange("p (m c) -> p m c", m=NM)[:, :, lcol:lcol + BLOC],
                        )
                        sig = cellpool.tile([128, 9 * BLOC], fp32, tag="sig")
                        nc.scalar.activation(sig[:], g[:, 0:9 * BLOC], AF.Sigmoid)
                        tg = cellpool.tile([128, NKH * BLOC], fp32, tag="tg")
                        nc.scalar.activation(tg[:], g[:, 9 * BLOC:12 * BLOC], AF.Tanh)
                        t1 = cellpool.tile([128, NKH * BLOC], fp32, tag="t1")
                        nc.vector.tensor_mul(t1[:], sig[:, 0:NKH * BLOC], tg[:])
                        t2 = cellpool.tile([128, NKH * BLOC], fp32, tag="t2")
                        nc.vector.tensor_mul(t2[:], sig[:, NKH * BLOC:6 * BLOC], c_prev[:])
                        cnew = cellpool.tile([128, NKH * BLOC], fp32, tag="cst")
                        nc.vector.tensor_add(cnew[:], t1[:], t2[:])
                        th = cellpool.tile([128, NKH * BLOC], fp32, tag="th")
                        nc.scalar.activation(th[:], cnew[:], AF.Tanh)
                        hv = h_sb.rearrange("p (h s c) -> p h s c", h=NKH, c=BLOC)
                        nc.vector.tensor_mul(
                            hv[:, :, ws, :],
                            sig[:, 6 * BLOC:9 * BLOC].rearrange("p (h c) -> p h c", h=NKH),
                            th.rearrange("p (h c) -> p h c", h=NKH),
                        )
                        c_prev = cnew

                    # ---- store chunk h to DRAM ----
                    dv = hT[u].rearrange("(h p) t -> p h t", p=128)
                    sv2 = h_sb.rearrange("p (h s c) -> p h s c", h=NKH, c=BLOC)
                    nc.sync.dma_start(
                        out=dv[:, :, cb:cb + TC * BLOC]
                        .rearrange("p h (s c) -> p h s c", c=BLOC),
                        in_=sv2[:, :, 1:TC + 1, :],
                    )

            scan_unit("l0f", [(xT, NKD)], True)
            scan_unit("l0b", [(xT, NKD)], False)
            scan_unit("l1f", [(hT["l0f"], NKH), (hT["l0b"], NKH)], True)
            scan_unit("l1b", [(hT["l0f"], NKH), (hT["l0b"], NKH)], False)

        # ---------------- FC + LayerNorm ----------------
        with (
            tc.tile_pool(name="fcpool", bufs=2) as fcpool,
            tc.tile_pool(name="fsing", bufs=1) as fsing,
            tc.tile_pool(name="fpsum", bufs=2, space="PSUM") as fpsum,
            tc.tile_pool(name="tpsum", bufs=2, space="PSUM") as tpsum,
        ):
            fcw_sb = fsing.tile([128, NKD * K], fp32)
            nc.sync.dma_start(out=fcw_sb[:], in_=fcw[:])
            fcb_sb = fsing.tile([K, 1], fp32)
            nc.sync.dma_start(out=fcb_sb[:], in_=fcb[:])
            id9_sb = fsing.tile([K, K], fp32)
            nc.sync.dma_start(out=id9_sb[:], in_=id9[:])
            lng_sb = fsing.tile([128, 32 * K], fp32)
            nc.sync.dma_start(out=lng_sb[:], in_=lng[:])
            lnb_sb = fsing.tile([128, 32 * K], fp32)
            nc.sync.dma_start(out=lnb_sb[:], in_=lnb[:])
            ftT = fsing.tile([K, CT], fp32)

            NB = 512
            for n in range(CT // NB):
                xf = fcpool.tile([128, NKD * NB], fp32, tag="xf")
                for di, u in enumerate(("l1f", "l1b")):
                    sv = hT[u].rearrange("(k p) t -> p k t", p=128)
                    nc.sync.dma_start(
                        out=xf[:, di * NKH * NB:(di + 1) * NKH * NB]
                        .rearrange("p (k c) -> p k c", k=NKH),
                        in_=sv[:, :, n * NB:(n + 1) * NB],
                    )
                psf = fpsum.tile([K, NB], fp32, tag="psf")
                for k in range(NKD):
                    nc.tensor.matmul(
                        psf[:], fcw_sb[:, k * K:(k + 1) * K],
                        xf[:, k * NB:(k + 1) * NB],
                        start=(k == 0), stop=(k == NKD - 1),
                    )
                nc.vector.tensor_scalar_add(ftT[:, n * NB:(n + 1) * NB], psf[:], fcb_sb[:])

            fst = fsing.tile([128, 32 * K], fp32)
            for blk in range(32):
                pst = tpsum.tile([128, K], fp32, tag="pst")
                nc.tensor.transpose(pst[:], ftT[:, blk * 128:(blk + 1) * 128], id9_sb[:])
                if blk % 2 == 0:
                    nc.vector.tensor_copy(fst[:, blk * K:(blk + 1) * K], pst[:])
                else:
                    nc.scalar.activation(fst[:, blk * K:(blk + 1) * K], pst[:], AF.Identity)

            f3 = fst.rearrange("p (t k) -> p t k", k=K)
            mu = fsing.tile([128, 32], fp32)
            nc.vector.reduce_sum(mu[:], f3, axis=AX.X)
            nc.scalar.activation(mu[:], mu[:], AF.Copy, scale=1.0 / K)
            cen = fsing.tile([128, 32 * K], fp32)
            nc.vector.tensor_sub(cen.rearrange("p (t k) -> p t k", k=K), f3,
                                 bc(f3, mu.rearrange("p (t k) -> p t k", k=1)))
            sq = fsing.tile([128, 32 * K], fp32)
            c3 = cen.rearrange("p (t k) -> p t k", k=K)
            nc.vector.tensor_mul(sq.rearrange("p (t k) -> p t k", k=K), c3, c3)
            var = fsing.tile([128, 32], fp32)
            nc.vector.reduce_sum(var[:], sq.rearrange("p (t k) -> p t k", k=K), axis=AX.X)
            sd = fsing.tile([128, 32], fp32)
            nc.vector.tensor_scalar(sd[:], var[:], 1.0 / K, 1e-5,
                                    op0=ALU.mult, op1=ALU.add)
            nc.scalar.activation(sd[:], sd[:], AF.Sqrt)
            rstd = fsing.tile([128, 32], fp32)
            nc.vector.reciprocal(rstd[:], sd[:])
            nrm = fsing.tile([128, 32 * K], fp32)
            nc.vector.tensor_mul(nrm.rearrange("p (t k) -> p t k", k=K), c3,
                                 bc(c3, rstd.rearrange("p (t k) -> p t k", k=1)))
            nc.vector.tensor_mul(nrm[:], nrm[:], lng_sb[:])
            nc.vector.tensor_add(nrm[:], nrm[:], lnb_sb[:])
            nc.sync.dma_start(
                out=feats_d.rearrange("(blk p) k -> p blk k", p=128),
                in_=nrm.rearrange("p (t k) -> p t k", k=K),
            )

        # ---------------- CRF (sequential) ----------------
        with (
            tc.tile_pool(name="csing", bufs=1) as csing,
            tc.tile_pool(name="cpool", bufs=3) as cpool,
        ):
            cf = csing.tile([BLOC, T * K], fp32)
            nc.sync.dma_start(out=cf.rearrange("b (t k) -> b t k", k=K),
                              in_=feats_d.rearrange("(t b) k -> b t k", b=BLOC))
            tr_sb = csing.tile([BLOC, K * K], fp32)
            nc.sync.dma_start(out=tr_sb[:], in_=trep[:])
            i81_sb = csing.tile([BLOC, K * K], fp32)
            nc.sync.dma_start(out=i81_sb[:], in_=io81[:])
            i9_sb = csing.tile([BLOC, K], fp32)
            nc.sync.dma_start(out=i9_sb[:], in_=io9[:])
            la = cpool.tile([BLOC, K], fp32, tag="la")
            nc.sync.dma_start(out=la[:], in_=la0p[:])
            ld = cpool.tile([BLOC, K], fp32, tag="ld")
            nc.sync.dma_start(out=ld[:], in_=la0p[:])
            psi = csing.tile([BLOC, (T - 1) * K], fp32)
            path = csing.tile([BLOC, T], fp32)

            ALU_ = ALU
            for t in range(1, T):
                ft = cf[:, t * K:(t + 1) * K]
                # forward (log-semiring)
                sf = cpool.tile([BLOC, K * K], fp32, tag="sf")
                trv = tr_sb.rearrange("b (i k) -> b i k", k=K)
                nc.vector.tensor_add(sf.rearrange("b (i k) -> b i k", k=K),
                                     trv, bc(trv, la.rearrange("b (i k) -> b i k", i=1)))
                mf = cpool.tile([BLOC, K], fp32, tag="mf")
                nc.vector.reduce_max(mf[:], sf.rearrange("b (i k) -> b i k", k=K),
                                     axis=AX.X, negate=True)
                ef = cpool.tile([BLOC, K * K], fp32, tag="ef")
                sfv = sf.rearrange("b (i k) -> b i k", k=K)
                nc.vector.tensor_add(ef.rearrange("b (i k) -> b i k", k=K),
                                     sfv, bc(sfv, mf.rearrange("b (i k) -> b i k", k=1)))
                nc.scalar.activation(ef[:], ef[:], AF.Exp)
                ssum = cpool.tile([BLOC, K], fp32, tag="ssum")
                nc.vector.reduce_sum(ssum[:], ef.rearrange("b (i k) -> b i k", k=K),
                                     axis=AX.X)
                lg = cpool.tile([BLOC, K], fp32, tag="lg")
                nc.scalar.activation(lg[:], ssum[:], AF.Ln)
                la2 = cpool.tile([BLOC, K], fp32, tag="la2")
                nc.vector.tensor_sub(la2[:], lg[:], mf[:])
                la = cpool.tile([BLOC, K], fp32, tag="la")
                nc.vector.tensor_add(la[:], la2[:], ft)
                # viterbi (max-plus)
                sv = cpool.tile([BLOC, K * K], fp32, tag="sv")
                nc.vector.tensor_add(sv.rearrange("b (i k) -> b i k", k=K),
                                     trv, bc(trv, ld.rearrange("b (i k) -> b i k", i=1)))
                mv = cpool.tile([BLOC, K], fp32, tag="mv")
                nc.vector.reduce_max(mv[:], sv.rearrange("b (i k) -> b i k", k=K),
                                     axis=AX.X)
                ld = cpool.tile([BLOC, K], fp32, tag="ld")
                nc.vector.tensor_add(ld[:], mv[:], ft)
                eq = cpool.tile([BLOC, K * K], fp32, tag="eq")
                svv = sv.rearrange("b (i k) -> b i k", k=K)
                nc.vector.tensor_tensor(eq.rearrange("b (i k) -> b i k", k=K),
                                        svv, bc(svv, mv.rearrange("b (i k) -> b i k", k=1)),
                                        op=ALU_.is_equal)
                sel = cpool.tile([BLOC, K * K], fp32, tag="sel")
                nc.vector.tensor_mul(sel[:], eq[:], i81_sb[:])
                nc.vector.tensor_reduce(psi[:, (t - 1) * K:t * K],
                                        sel.rearrange("b (i k) -> b i k", k=K),
                                        axis=AX.X, op=ALU_.min)

            # score = logsumexp(la)
            nm = cpool.tile([BLOC, 1], fp32, tag="nm")
            nc.vector.reduce_max(nm[:], la[:], axis=AX.X, negate=True)
            ex = cpool.tile([BLOC, K], fp32, tag="ex")
            nc.scalar.activation(ex[:], la[:], AF.Exp, bias=nm[:])
            sm = cpool.tile([BLOC, 1], fp32, tag="sm")
            nc.vector.reduce_sum(sm[:], ex[:], axis=AX.X)
            lgs = cpool.tile([BLOC, 1], fp32, tag="lgs")
            nc.scalar.activation(lgs[:], sm[:], AF.Ln)
            sc = cpool.tile([BLOC, 1], fp32, tag="sc")
            nc.vector.tensor_sub(sc[:], lgs[:], nm[:])
            nc.sync.dma_start(out=out_score[:], in_=sc[:])

            # backtrack
            ml = cpool.tile([BLOC, 1], fp32, tag="ml")
            nc.vector.reduce_max(ml[:], ld[:], axis=AX.X)
            eql = cpool.tile([BLOC, K], fp32, tag="eql")
            nc.vector.tensor_tensor(eql[:], ld[:], bc(ld[:], ml[:]),
                                    op=ALU_.is_equal)
            sell = cpool.tile([BLOC, K], fp32, tag="sell")
            nc.vector.tensor_mul(sell[:], eql[:], i9_sb[:])
            nc.vector.tensor_reduce(path[:, T - 1:T], sell[:], axis=AX.X, op=ALU_.min)
            oh = cpool.tile([BLOC, K], fp32, tag="oh")
            nc.vector.tensor_tensor(oh[:], i9_sb[:], bc(i9_sb[:], path[:, T - 1:T]),
                                    op=ALU_.is_equal)
            for t in range(T - 2, -1, -1):
                sel2 = cpool.tile([BLOC, K], fp32, tag="sel2")
                nc.vector.tensor_mul(sel2[:], oh[:], psi[:, t * K:(t + 1) * K])
                nc.vector.tensor_reduce(path[:, t:t + 1], sel2[:], axis=AX.X, op=ALU_.min)
                oh = cpool.tile([BLOC, K], fp32, tag="oh")
                nc.vector.tensor_tensor(oh[:], i9_sb[:], bc(i9_sb[:], path[:, t:t + 1]),
                                        op=ALU_.is_equal)
            pfix = csing.tile([BLOC, T], fp32)
            nc.vector.tensor_scalar_add(pfix[:], path[:], BIGF)
            nc.sync.dma_start(out=out_path[:], in_=pfix[:])

    _split_waits(nc, mybir)
    return nc


def _split_waits(nc, mybir):
    """The walrus build here accepts at most one sync wait per instruction;
    hoist extras onto standalone InstEventSemaphore entries just before, on
    the same engine stream (raw-bass wait_ge style)."""
    nid = 0
    for f in nc.m.functions:
        for b in f.blocks:
            arr = list(b.instructions)
            out = []
            changed = False
            for ins in arr:
                si = ins.sync_info
                if si is not None and len(si.on_wait) > 1:
                    waits = list(si.on_wait)
                    for w in waits[:-1]:
                        nid += 1
                        ev = mybir.InstEventSemaphore(
                            name=f"I-wsplit-{nid}", ins=[], outs=[])
                        ev.engine = ins.engine
                        ev.sync_info = mybir.SyncInfo(on_wait=[w], on_update=[])
                        out.append(ev)
                    ins.sync_info = mybir.SyncInfo(
                        on_wait=[waits[-1]], on_update=list(si.on_update))
                    changed = True
                out.append(ins)
            if changed:
                b.instructions = out


def _prep_shared(lstm_params, fc_w, fc_b, ln_g, ln_b, transitions):
    """Host-side weight layout prep (pure reshapes, replicated to all cores)."""
    f32 = np.float32
    out = {}
    # gate reorder (i, f, g, o) -> (i, f, o, g)
    perm = np.concatenate([np.arange(0, 2 * H), np.arange(3 * H, 4 * H),
                           np.arange(2 * H, 3 * H)])
    for layer in range(2):
        for d in ("f", "b"):
            u = f"l{layer}{d}"
            wi = np.asarray(lstm_params[f"Wih_l{layer}{d}"], f32)[perm]      # [1536, 768]
            wh = np.asarray(lstm_params[f"Whh_l{layer}{d}"], f32)[perm]      # [1536, 384]
            bi = (np.asarray(lstm_params[f"bih_l{layer}{d}"], f32)
                  + np.asarray(lstm_params[f"bhh_l{layer}{d}"], f32))[perm]  # [1536]
            wiT = np.ascontiguousarray(wi.T)   # [768, 1536]
            whT = np.ascontiguousarray(wh.T)   # [384, 1536]
            out[f"wih_{u}"] = np.ascontiguousarray(
                wiT.reshape(NKD, 128, NM, 128).transpose(1, 0, 2, 3).reshape(128, NKD * GD))
            out[f"whh_{u}"] = np.ascontiguousarray(
                whT.reshape(NKH, 128, NM, 128).transpose(1, 0, 2, 3).reshape(128, NKH * GD))
            out[f"bias_{u}"] = np.ascontiguousarray(bi.reshape(NM, 128).T)
    fcwT = np.ascontiguousarray(np.asarray(fc_w, f32).T)  # [768, 9]
    out["fcw"] = np.ascontiguousarray(
        fcwT.reshape(NKD, 128, K).transpose(1, 0, 2).reshape(128, NKD * K))
    out["fcb"] = np.asarray(fc_b, f32).reshape(K, 1)
    out["lng_rep"] = np.tile(np.asarray(ln_g, f32), (128, 32))
    out["lnb_rep"] = np.tile(np.asarray(ln_b, f32), (128, 32))
    out["trans_rep"] = np.tile(np.asarray(transitions, f32).reshape(1, K * K), (BLOC, 1))
    out["iota81"] = np.tile((np.arange(K, dtype=f32) - BIGF), (BLOC, K))
    out["iota9"] = np.tile((np.arange(K, dtype=f32) - BIGF), (BLOC, 1))
    la0 = np.full((BLOC, K), NEG, f32)
    la0[:, START] = 0.0
    out["la0"] = la0
    out["ident9"] = np.eye(K, dtype=f32)
    return out


def _run(inputs, trace=False):
    from concourse.bass_utils import run_bass_kernel_spmd

    if "nc" not in _cache:
        _cache["nc"] = _build_nc()
    nc = _cache["nc"]

    shared = _prep_shared(inputs["lstm_params"], inputs["fc_w"], inputs["fc_b"],
                          inputs["ln_g"], inputs["ln_b"], inputs["transitions"])
    embeds = np.asarray(inputs["embeds"], np.float32)
    in_maps = []
    for ci in range(NCORES):
        m = dict(shared)
        sh = embeds[ci * BLOC:(ci + 1) * BLOC]          # [8, 512, 768]
        m["xT"] = np.ascontiguousarray(sh.transpose(2, 1, 0).reshape(D, CT))
        in_maps.append(m)

    res = run_bass_kernel_spmd(nc, in_maps, list(range(NCORES)), trace=trace)
    score = np.concatenate([res.results[i]["out_score"][:, 0] for i in range(NCORES)])
    path = np.concatenate([res.results[i]["out_path"] for i in range(NCORES)], axis=0)
    path = np.rint(path).astype(np.int32)
    return (score, path), res


def kernel(**inputs):
    out, _ = _run(inputs, trace=False)
    return out


# revision 7
# speedup vs baseline: 1.0229x; 1.0229x over previous
"""Trainium2 Bass kernel for BiLSTM-CRF (B=64, T=512, D=768, H=384, K=9).

Sharding: 8-way data parallel over batch (b=8 per core). All compute on
device; host only reshapes/shards inputs and concatenates outputs.

Per-core layouts (everything "transposed": feature dim on partitions,
(t, batch) in the free dim) so elementwise LSTM-cell work uses all 128
partitions and the recurrent state needs no per-step transposes:
  xT      [768, T*b]   layer-0 input
  WxT     [12*128, cols] gate pre-activations (gate-chunk-major)
  hT      [384, T*b]   per-direction hidden history (DRAM staged)
  featsT  [9, T*b] -> transposed to [(t,b), 9] for LayerNorm + CRF
Gate order is host-remapped to (i, f, o, g) so sigmoid covers one
contiguous [128, 72] slab and tanh one [128, 24] slab per step.
CRF runs sequentially (T steps) on 8 partitions; Viterbi backpointers are
encoded as (argmax - 1e5) floats so ties resolve to the first index,
matching jnp.argmax.
"""

import numpy as np

B, T, D = 64, 512, 768
H = 384
K = 9
START = 7
NEG = -10000.0
BIGF = 100000.0  # index-encoding offset (exact in fp32 for small ints)
NCORES = 8
BLOC = B // NCORES  # 8
TC = 32             # time-chunk for the LSTM scans
NCH = T // TC
GD = 4 * H          # 1536
NM = GD // 128      # 12 gate chunks
NKD = D // 128      # 6 input contraction chunks
NKH = H // 128      # 3 recurrent contraction chunks
CT = T * BLOC       # 4096 columns

_cache = {}


def _build_nc():
    import concourse.bass as bass
    import concourse.mybir as mybir
    from concourse.tile import TileContext
    from concourse.bass import broadcast_tensor_aps

    def bc(full, small):
        _, s2 = broadcast_tensor_aps(full, small)
        return s2

    fp32 = mybir.dt.float32
    AF = mybir.ActivationFunctionType
    ALU = mybir.AluOpType
    AX = mybir.AxisListType

    nc = bass.Bass()

    def par(name, shape, out=False):
        return nc.declare_dram_parameter(name, list(shape), fp32, isOutput=out)

    xT = par("xT", [D, CT])
    wih = {u: par(f"wih_{u}", [128, NKD * GD]) for u in ("l0f", "l0b", "l1f", "l1b")}
    whh = {u: par(f"whh_{u}", [128, NKH * GD]) for u in ("l0f", "l0b", "l1f", "l1b")}
    bia = {u: par(f"bias_{u}", [128, NM]) for u in ("l0f", "l0b", "l1f", "l1b")}
    fcw = par("fcw", [128, NKD * K])
    fcb = par("fcb", [K, 1])
    lng = par("lng_rep", [128, 32 * K])
    lnb = par("lnb_rep", [128, 32 * K])
    trep = par("trans_rep", [BLOC, K * K])
    io81 = par("iota81", [BLOC, K * K])
    io9 = par("iota9", [BLOC, K])
    la0p = par("la0", [BLOC, K])
    id9 = par("ident9", [K, K])

    hT = {u: nc.dram_tensor(f"hT_{u}", [H, CT], fp32) for u in ("l0f", "l0b", "l1f", "l1b")}
    feats_d = nc.dram_tensor("feats_d", [T * BLOC, K], fp32)

    out_score = par("out_score", [BLOC, 1], out=True)
    out_path = par("out_path", [BLOC, T], out=True)

    with TileContext(nc) as tc:
        # ---------------- LSTM scans ----------------
        with (
            tc.tile_pool(name="wpool", bufs=1) as wpool,
            tc.tile_pool(name="xpool", bufs=2) as xpool,
            tc.tile_pool(name="wxpool", bufs=1) as wxpool,
            tc.tile_pool(name="hpool", bufs=2) as hpool,
            tc.tile_pool(name="cellpool", bufs=2) as cellpool,
            tc.tile_pool(name="gpsum", bufs=2, space="PSUM") as gpsum,
            tc.tile_pool(name="rpsum", bufs=2, space="PSUM") as rpsum,
        ):
            def scan_pair(units):
                """Run two independent (direction) scans in lockstep so each
                stream's ACT/DVE chain overlaps the other's PE matmuls."""
                W = {}
                for (u, srcs, fwd) in units:
                    wih_sb = wpool.tile([128, NKD * GD], fp32, tag=f"wih_{u[-1]}")
                    nc.sync.dma_start(out=wih_sb[:], in_=wih[u][:])
                    whh_sb = wpool.tile([128, NKH * GD], fp32, tag=f"whh_{u[-1]}")
                    nc.sync.dma_start(out=whh_sb[:], in_=whh[u][:])
                    bias_sb = wpool.tile([128, NM], fp32, tag=f"bias_{u[-1]}")
                    nc.sync.dma_start(out=bias_sb[:], in_=bia[u][:])
                    W[u] = dict(wih=wih_sb, whh=whh_sb, bias=bias_sb,
                                srcs=srcs, fwd=fwd, hprev=None, cprev=None, wx=None)

                for c in range(NCH):
                    for (u, srcs, fwd) in units:
                        st = W[u]
                        cb = (c if fwd else (NCH - 1 - c)) * TC * BLOC
                        xt = xpool.tile([128, NKD * TC * BLOC], fp32, tag=f"xc_{u[-1]}")
                        col = 0
                        for (srcd, nk) in srcs:
                            sv = srcd.rearrange("(k p) t -> p k t", p=128)
                            nc.sync.dma_start(
                                out=xt[:, col * TC * BLOC:(col + nk) * TC * BLOC]
                                .rearrange("p (k c) -> p k c", k=nk),
                                in_=sv[:, :, cb:cb + TC * BLOC],
                            )
                            col += nk
                        wx = wxpool.tile([128, NM * TC * BLOC], fp32, tag=f"wx_{u[-1]}")
                        st["wx"] = wx
                        st["cb"] = cb
                        for m in range(NM):
                            ps = gpsum.tile([128, TC * BLOC], fp32, tag=f"gps_{u[-1]}")
                            for k in range(NKD):
                                nc.tensor.matmul(
                                    ps[:],
                                    st["wih"][:, k * GD + m * 128:k * GD + (m + 1) * 128],
                                    xt[:, k * TC * BLOC:(k + 1) * TC * BLOC],
                                    start=(k == 0), stop=(k == NKD - 1),
                                )
                            dst = wx[:, m * TC * BLOC:(m + 1) * TC * BLOC]
                            if m % 2 == 0:
                                nc.vector.tensor_scalar_add(dst, ps[:], st["bias"][:, m:m + 1])
                            else:
                                nc.scalar.activation(dst, ps[:], AF.Identity,
                                                     bias=st["bias"][:, m:m + 1])

                        SL = TC + 2
                        h_sb = hpool.tile([128, NKH * SL * BLOC], fp32, tag=f"hsb_{u[-1]}")
                        if c == 0:
                            s0 = 0 if fwd else SL - 1
                            z = h_sb.rearrange("p (h s c) -> p h s c", h=NKH, c=BLOC)
                            nc.vector.memset(z[:, :, s0, :], 0.0)
                            cpv = cellpool.tile([128, NKH * BLOC], fp32, tag=f"cst_{u[-1]}")
                            nc.vector.memset(cpv[:], 0.0)
                            st["cprev"] = cpv
                        else:
                            pv = st["hprev"].rearrange("p (h s c) -> p h s c", h=NKH, c=BLOC)
                            zv = h_sb.rearrange("p (h s c) -> p h s c", h=NKH, c=BLOC)
                            if fwd:
                                nc.vector.tensor_copy(zv[:, :, 0, :], pv[:, :, TC, :])
                            else:
                                nc.vector.tensor_copy(zv[:, :, SL - 1, :], pv[:, :, 1, :])
                        st["hprev"] = h_sb

                    for j in range(TC):
                        for (u, srcs, fwd) in units:
                            st = W[u]
                            h_sb = st["hprev"]
                            wx = st["wx"]
                            SL = TC + 2
                            rs = j if fwd else (SL - 1 - j)
                            ws = j + 1 if fwd else (SL - 2 - j)
                            lcol = (j if fwd else (TC - 1 - j)) * BLOC
                            ps = rpsum.tile([128, NM * BLOC], fp32, tag=f"rps_{u[-1]}")
                            for m in range(NM):
                                for k in range(NKH):
                                    nc.tensor.matmul(
                                        ps[:, m * BLOC:(m + 1) * BLOC],
                                        st["whh"][:, k * GD + m * 128:k * GD + (m + 1) * 128],
                                        h_sb[:, (k * SL + rs) * BLOC:(k * SL + rs + 1) * BLOC],
                                        start=(k == 0), stop=(k == NKH - 1),
                                    )
                            g = cellpool.tile([128, NM * BLOC], fp32, tag=f"g_{u[-1]}")
                            nc.vector.tensor_add(
                                g.rearrange("p (m c) -> p m c", m=NM),
                                ps.rearrange("p (m c) -> p m c", m=NM),
                                wx.rearrange("p (m c) -> p m c", m=NM)[:, :, lcol:lcol + BLOC],
                            )
                            sig = cellpool.tile([128, 9 * BLOC], fp32, tag=f"sig_{u[-1]}")
                            nc.scalar.activation(sig[:], g[:, 0:9 * BLOC], AF.Sigmoid)
                            tg = cellpool.tile([128, NKH * BLOC], fp32, tag=f"tg_{u[-1]}")
                            nc.scalar.activation(tg[:], g[:, 9 * BLOC:12 * BLOC], AF.Tanh)
                            t1 = cellpool.tile([128, NKH * BLOC], fp32, tag=f"t1_{u[-1]}")
                            nc.vector.tensor_mul(t1[:], sig[:, 0:NKH * BLOC], tg[:])
                            t2 = cellpool.tile([128, NKH * BLOC], fp32, tag=f"t2_{u[-1]}")
                            nc.vector.tensor_mul(t2[:], sig[:, NKH * BLOC:6 * BLOC], st["cprev"][:])
                            cnew = cellpool.tile([128, NKH * BLOC], fp32, tag=f"cst_{u[-1]}")
                            nc.vector.tensor_add(cnew[:], t1[:], t2[:])
                            th = cellpool.tile([128, NKH * BLOC], fp32, tag=f"th_{u[-1]}")
                            nc.scalar.activation(th[:], cnew[:], AF.Tanh)
                            hv = h_sb.rearrange("p (h s c) -> p h s c", h=NKH, c=BLOC)
                            nc.vector.tensor_mul(
                                hv[:, :, ws, :],
                                sig[:, 6 * BLOC:9 * BLOC].rearrange("p (h c) -> p h c", h=NKH),
                                th.rearrange("p (h c) -> p h c", h=NKH),
                            )
                            st["cprev"] = cnew

                    for (u, srcs, fwd) in units:
                        st = W[u]
                        dv = hT[u].rearrange("(h p) t -> p h t", p=128)
                        sv2 = st["hprev"].rearrange("p (h s c) -> p h s c", h=NKH, c=BLOC)
                        nc.sync.dma_start(
                            out=dv[:, :, st["cb"]:st["cb"] + TC * BLOC]
                            .rearrange("p h (s c) -> p h s c", c=BLOC),
                            in_=sv2[:, :, 1:TC + 1, :],
                        )

            scan_pair([("l0f", [(xT, NKD)], True), ("l0b", [(xT, NKD)], False)])
            scan_pair([("l1f", [(hT["l0f"], NKH), (hT["l0b"], NKH)], True),
                       ("l1b", [(hT["l0f"], NKH), (hT["l0b"], NKH)], False)])

        # ---------------- FC + LayerNorm ----------------
        with (
            tc.tile_pool(name="fcpool", bufs=2) as fcpool,
            tc.tile_pool(name="fsing", bufs=1) as fsing,
            tc.tile_pool(name="fpsum", bufs=2, space="PSUM") as fpsum,
            tc.tile_pool(name="tpsum", bufs=2, space="PSUM") as tpsum,
        ):
            fcw_sb = fsing.tile([128, NKD * K], fp32)
            nc.sync.dma_start(out=fcw_sb[:], in_=fcw[:])
            fcb_sb = fsing.tile([K, 1], fp32)
            nc.sync.dma_start(out=fcb_sb[:], in_=fcb[:])
            id9_sb = fsing.tile([K, K], fp32)
            nc.sync.dma_start(out=id9_sb[:], in_=id9[:])
            lng_sb = fsing.tile([128, 32 * K], fp32)
            nc.sync.dma_start(out=lng_sb[:], in_=lng[:])
            lnb_sb = fsing.tile([128, 32 * K], fp32)
            nc.sync.dma_start(out=lnb_sb[:], in_=lnb[:])
            ftT = fsing.tile([K, CT], fp32)

            NB = 512
            for n in range(CT // NB):
                xf = fcpool.tile([128, NKD * NB], fp32, tag="xf")
                for di, u in enumerate(("l1f", "l1b")):
                    sv = hT[u].rearrange("(k p) t -> p k t", p=128)
                    nc.sync.dma_start(
                        out=xf[:, di * NKH * NB:(di + 1) * NKH * NB]
                        .rearrange("p (k c) -> p k c", k=NKH),
                        in_=sv[:, :, n * NB:(n + 1) * NB],
                    )
                psf = fpsum.tile([K, NB], fp32, tag="psf")
                for k in range(NKD):
                    nc.tensor.matmul(
                        psf[:], fcw_sb[:, k * K:(k + 1) * K],
                        xf[:, k * NB:(k + 1) * NB],
                        start=(k == 0), stop=(k == NKD - 1),
                    )
                nc.vector.tensor_scalar_add(ftT[:, n * NB:(n + 1) * NB], psf[:], fcb_sb[:])

            fst = fsing.tile([128, 32 * K], fp32)
            for blk in range(32):
                pst = tpsum.tile([128, K], fp32, tag="pst")
                nc.tensor.transpose(pst[:], ftT[:, blk * 128:(blk + 1) * 128], id9_sb[:])
                if blk % 2 == 0:
                    nc.vector.tensor_copy(fst[:, blk * K:(blk + 1) * K], pst[:])
                else:
                    nc.scalar.activation(fst[:, blk * K:(blk + 1) * K], pst[:], AF.Identity)

            f3 = fst.rearrange("p (t k) -> p t k", k=K)
            mu = fsing.tile([128, 32], fp32)
            nc.vector.reduce_sum(mu[:], f3, axis=AX.X)
            nc.scalar.activation(mu[:], mu[:], AF.Copy, scale=1.0 / K)
            cen = fsing.tile([128, 32 * K], fp32)
            nc.vector.tensor_sub(cen.rearrange("p (t k) -> p t k", k=K), f3,
                                 bc(f3, mu.rearrange("p (t k) -> p t k", k=1)))
            sq = fsing.tile([128, 32 * K], fp32)
            c3 = cen.rearrange("p (t k) -> p t k", k=K)
            nc.vector.tensor_mul(sq.rearrange("p (t k) -> p t k", k=K), c3, c3)
            var = fsing.tile([128, 32], fp32)
            nc.vector.reduce_sum(var[:], sq.rearrange("p (t k) -> p t k", k=K), axis=AX.X)
            sd = fsing.tile([128, 32], fp32)
            nc.vector.tensor_scalar(sd[:], var[:], 1.0 / K, 1e-5,
                                    op0=ALU.mult, op1=ALU.add)
            nc.scalar.activation(sd[:], sd[:], AF.Sqrt)
            rstd = fsing.tile([128, 32], fp32)
            nc.vector.reciprocal(rstd[:], sd[:])
            nrm = fsing.tile([128, 32 * K], fp32)
            nc.vector.tensor_mul(nrm.rearrange("p (t k) -> p t k", k=K), c3,
                                 bc(c3, rstd.rearrange("p (t k) -> p t k", k=1)))
            nc.vector.tensor_mul(nrm[:], nrm[:], lng_sb[:])
            nc.vector.tensor_add(nrm[:], nrm[:], lnb_sb[:])
            nc.sync.dma_start(
                out=feats_d.rearrange("(blk p) k -> p blk k", p=128),
                in_=nrm.rearrange("p (t k) -> p t k", k=K),
            )

        # ---------------- CRF (sequential) ----------------
        with (
            tc.tile_pool(name="csing", bufs=1) as csing,
            tc.tile_pool(name="cpool", bufs=3) as cpool,
        ):
            cf = csing.tile([BLOC, T * K], fp32)
            nc.sync.dma_start(out=cf.rearrange("b (t k) -> b t k", k=K),
                              in_=feats_d.rearrange("(t b) k -> b t k", b=BLOC))
            tr_sb = csing.tile([BLOC, K * K], fp32)
            nc.sync.dma_start(out=tr_sb[:], in_=trep[:])
            i81_sb = csing.tile([BLOC, K * K], fp32)
            nc.sync.dma_start(out=i81_sb[:], in_=io81[:])
            i9_sb = csing.tile([BLOC, K], fp32)
            nc.sync.dma_start(out=i9_sb[:], in_=io9[:])
            la = cpool.tile([BLOC, K], fp32, tag="la")
            nc.sync.dma_start(out=la[:], in_=la0p[:])
            ld = cpool.tile([BLOC, K], fp32, tag="ld")
            nc.sync.dma_start(out=ld[:], in_=la0p[:])
            psi = csing.tile([BLOC, (T - 1) * K], fp32)
            path = csing.tile([BLOC, T], fp32)

            ALU_ = ALU
            for t in range(1, T):
                ft = cf[:, t * K:(t + 1) * K]
                # forward (log-semiring)
                sf = cpool.tile([BLOC, K * K], fp32, tag="sf")
                trv = tr_sb.rearrange("b (i k) -> b i k", k=K)
                nc.vector.tensor_add(sf.rearrange("b (i k) -> b i k", k=K),
                                     trv, bc(trv, la.rearrange("b (i k) -> b i k", i=1)))
                mf = cpool.tile([BLOC, K], fp32, tag="mf")
                nc.vector.reduce_max(mf[:], sf.rearrange("b (i k) -> b i k", k=K),
                                     axis=AX.X, negate=True)
                ef = cpool.tile([BLOC, K * K], fp32, tag="ef")
                sfv = sf.rearrange("b (i k) -> b i k", k=K)
                nc.vector.tensor_add(ef.rearrange("b (i k) -> b i k", k=K),
                                     sfv, bc(sfv, mf.rearrange("b (i k) -> b i k", k=1)))
                nc.scalar.activation(ef[:], ef[:], AF.Exp)
                ssum = cpool.tile([BLOC, K], fp32, tag="ssum")
                nc.vector.reduce_sum(ssum[:], ef.rearrange("b (i k) -> b i k", k=K),
                                     axis=AX.X)
                lg = cpool.tile([BLOC, K], fp32, tag="lg")
                nc.scalar.activation(lg[:], ssum[:], AF.Ln)
                la2 = cpool.tile([BLOC, K], fp32, tag="la2")
                nc.vector.tensor_sub(la2[:], lg[:], mf[:])
                la = cpool.tile([BLOC, K], fp32, tag="la")
                nc.vector.tensor_add(la[:], la2[:], ft)
                # viterbi (max-plus)
                sv = cpool.tile([BLOC, K * K], fp32, tag="sv")
                nc.vector.tensor_add(sv.rearrange("b (i k) -> b i k", k=K),
                                     trv, bc(trv, ld.rearrange("b (i k) -> b i k", i=1)))
                mv = cpool.tile([BLOC, K], fp32, tag="mv")
                nc.vector.reduce_max(mv[:], sv.rearrange("b (i k) -> b i k", k=K),
                                     axis=AX.X)
                ld = cpool.tile([BLOC, K], fp32, tag="ld")
                nc.vector.tensor_add(ld[:], mv[:], ft)
                eq = cpool.tile([BLOC, K * K], fp32, tag="eq")
                svv = sv.rearrange("b (i k) -> b i k", k=K)
                nc.vector.tensor_tensor(eq.rearrange("b (i k) -> b i k", k=K),
                                        svv, bc(svv, mv.rearrange("b (i k) -> b i k", k=1)),
                                        op=ALU_.is_equal)
                sel = cpool.tile([BLOC, K * K], fp32, tag="sel")
                nc.vector.tensor_mul(sel[:], eq[:], i81_sb[:])
                nc.vector.tensor_reduce(psi[:, (t - 1) * K:t * K],
                                        sel.rearrange("b (i k) -> b i k", k=K),
                                        axis=AX.X, op=ALU_.min)

            # score = logsumexp(la)
            nm = cpool.tile([BLOC, 1], fp32, tag="nm")
            nc.vector.reduce_max(nm[:], la[:], axis=AX.X, negate=True)
            ex = cpool.tile([BLOC, K], fp32, tag="ex")
            nc.scalar.activation(ex[:], la[:], AF.Exp, bias=nm[:])
            sm = cpool.tile([BLOC, 1], fp32, tag="sm")
            nc.vector.reduce_sum(sm[:], ex[:], axis=AX.X)
            lgs = cpool.tile([BLOC, 1], fp32, tag="lgs")
            nc.scalar.activation(lgs[:], sm[:], AF.Ln)
            sc = cpool.tile([BLOC, 1], fp32, tag="sc")
            nc.vector.tensor_sub(sc[:], lgs[:], nm[:])
            nc.sync.dma_start(out=out_score[:], in_=sc[:])

            # backtrack
            ml = cpool.tile([BLOC, 1], fp32, tag="ml")
            nc.vector.reduce_max(ml[:], ld[:], axis=AX.X)
            eql = cpool.tile([BLOC, K], fp32, tag="eql")
            nc.vector.tensor_tensor(eql[:], ld[:], bc(ld[:], ml[:]),
                                    op=ALU_.is_equal)
            sell = cpool.tile([BLOC, K], fp32, tag="sell")
            nc.vector.tensor_mul(sell[:], eql[:], i9_sb[:])
            nc.vector.tensor_reduce(path[:, T - 1:T], sell[:], axis=AX.X, op=ALU_.min)
            oh = cpool.tile([BLOC, K], fp32, tag="oh")
            nc.vector.tensor_tensor(oh[:], i9_sb[:], bc(i9_sb[:], path[:, T - 1:T]),
                                    op=ALU_.is_equal)
            for t in range(T - 2, -1, -1):
                sel2 = cpool.tile([BLOC, K], fp32, tag="sel2")
                nc.vector.tensor_mul(sel2[:], oh[:], psi[:, t * K:(t + 1) * K])
                nc.vector.tensor_reduce(path[:, t:t + 1], sel2[:], axis=AX.X, op=ALU_.min)
                oh = cpool.tile([BLOC, K], fp32, tag="oh")
                nc.vector.tensor_tensor(oh[:], i9_sb[:], bc(i9_sb[:], path[:, t:t + 1]),
                                        op=ALU_.is_equal)
            pfix = csing.tile([BLOC, T], fp32)
            nc.vector.tensor_scalar_add(pfix[:], path[:], BIGF)
            nc.sync.dma_start(out=out_path[:], in_=pfix[:])

    _split_waits(nc, mybir)
    return nc


def _split_waits(nc, mybir):
    """The walrus build here accepts at most one sync wait per instruction;
    hoist extras onto standalone InstEventSemaphore entries just before, on
    the same engine stream (raw-bass wait_ge style)."""
    nid = 0
    for f in nc.m.functions:
        for b in f.blocks:
            arr = list(b.instructions)
            out = []
            changed = False
            for ins in arr:
                si = ins.sync_info
                if si is not None and len(si.on_wait) > 1:
                    waits = list(si.on_wait)
                    for w in waits[:-1]:
                        nid += 1
                        ev = mybir.InstEventSemaphore(
                            name=f"I-wsplit-{nid}", ins=[], outs=[])
                        ev.engine = ins.engine
                        ev.sync_info = mybir.SyncInfo(on_wait=[w], on_update=[])
                        out.append(ev)
                    ins.sync_info = mybir.SyncInfo(
                        on_wait=[waits[-1]], on_update=list(si.on_update))
                    changed = True
                out.append(ins)
            if changed:
                b.instructions = out


def _prep_shared(lstm_params, fc_w, fc_b, ln_g, ln_b, transitions):
    """Host-side weight layout prep (pure reshapes, replicated to all cores)."""
    f32 = np.float32
    out = {}
    # gate reorder (i, f, g, o) -> (i, f, o, g)
    perm = np.concatenate([np.arange(0, 2 * H), np.arange(3 * H, 4 * H),
                           np.arange(2 * H, 3 * H)])
    for layer in range(2):
        for d in ("f", "b"):
            u = f"l{layer}{d}"
            wi = np.asarray(lstm_params[f"Wih_l{layer}{d}"], f32)[perm]      # [1536, 768]
            wh = np.asarray(lstm_params[f"Whh_l{layer}{d}"], f32)[perm]      # [1536, 384]
            bi = (np.asarray(lstm_params[f"bih_l{layer}{d}"], f32)
                  + np.asarray(lstm_params[f"bhh_l{layer}{d}"], f32))[perm]  # [1536]
            wiT = np.ascontiguousarray(wi.T)   # [768, 1536]
            whT = np.ascontiguousarray(wh.T)   # [384, 1536]
            out[f"wih_{u}"] = np.ascontiguousarray(
                wiT.reshape(NKD, 128, NM, 128).transpose(1, 0, 2, 3).reshape(128, NKD * GD))
            out[f"whh_{u}"] = np.ascontiguousarray(
                whT.reshape(NKH, 128, NM, 128).transpose(1, 0, 2, 3).reshape(128, NKH * GD))
            out[f"bias_{u}"] = np.ascontiguousarray(bi.reshape(NM, 128).T)
    fcwT = np.ascontiguousarray(np.asarray(fc_w, f32).T)  # [768, 9]
    out["fcw"] = np.ascontiguousarray(
        fcwT.reshape(NKD, 128, K).transpose(1, 0, 2).reshape(128, NKD * K))
    out["fcb"] = np.asarray(fc_b, f32).reshape(K, 1)
    out["lng_rep"] = np.tile(np.asarray(ln_g, f32), (128, 32))
    out["lnb_rep"] = np.tile(np.asarray(ln_b, f32), (128, 32))
    out["trans_rep"] = np.tile(np.asarray(transitions, f32).reshape(1, K * K), (BLOC, 1))
    out["iota81"] = np.tile((np.arange(K, dtype=f32) - BIGF), (BLOC, K))
    out["iota9"] = np.tile((np.arange(K, dtype=f32) - BIGF), (BLOC, 1))
    la0 = np.full((BLOC, K), NEG, f32)
    la0[:, START] = 0.0
    out["la0"] = la0
    out["ident9"] = np.eye(K, dtype=f32)
    return out


def _run(inputs, trace=False):
    from concourse.bass_utils import run_bass_kernel_spmd

    if "nc" not in _cache:
        _cache["nc"] = _build_nc()
    nc = _cache["nc"]

    shared = _prep_shared(inputs["lstm_params"], inputs["fc_w"], inputs["fc_b"],
                          inputs["ln_g"], inputs["ln_b"], inputs["transitions"])
    embeds = np.asarray(inputs["embeds"], np.float32)
    in_maps = []
    for ci in range(NCORES):
        m = dict(shared)
        sh = embeds[ci * BLOC:(ci + 1) * BLOC]          # [8, 512, 768]
        m["xT"] = np.ascontiguousarray(sh.transpose(2, 1, 0).reshape(D, CT))
        in_maps.append(m)

    res = run_bass_kernel_spmd(nc, in_maps, list(range(NCORES)), trace=trace)
    score = np.concatenate([res.results[i]["out_score"][:, 0] for i in range(NCORES)])
    path = np.concatenate([res.results[i]["out_path"] for i in range(NCORES)], axis=0)
    path = np.rint(path).astype(np.int32)
    return (score, path), res


def kernel(**inputs):
    out, _ = _run(inputs, trace=False)
    return out
